# revision 13
# baseline (speedup 1.0000x reference)
"""DrugGNN segment-mean + linear embed, v4: all-PE DoubleRow design.

Architecture (per core, 2048 segs = 16 groups x 128 segs = 64 blocks x 32):
  - Host pads every segment count to a multiple of 8 ("slot rows" of 8
    nodes), snake-packs segments into 512 (core, block) bins of exactly 32
    segments each so every block has <= T*128 = 512 slot rows, and
    sigma-delta quantizes x on the fp8-e4m3 grid (error feedback makes
    per-segment sums exact to ~1 quant step).
  - Block slab layout [128p, T*512]: Q-group g occupies cols [g*512,
    (g+1)*512); its 8 tiles of 64 channels share ONE onehot pattern
    (row -> local seg), so each Q-group is a single DoubleRow matmul:
    lhsT = onehot [128, (0,2),(1,32)] fp8e4 (stride-0 k-tile share),
    rhs = slab [128, (64,2),(128,4),(1,64)], out = acc[strip:strip+32]
    with stride-0 free AP [(0,4),(1,64)] accumulating all 4 pairs into
    the same PSUM columns. 131ns per 1024 nodes measured.
  - Onehots built on DVE: is_equal(iota[128,T*32], srel bcast [(1,T),(0,32)]).
  - Epilogue per group: ACT scale (1/cnt) -> fp16 means + ones col, PE
    transpose, fp16 GEMM with [weight.T; bias], DMA out. Host un-permutes
    rows at the end.
"""
import numpy as np

N_NODES = 2_000_000
IN_CH = 64
OUT_CH = 128
NUM_GRAPHS = 16384
N_CORES = 8
P = 128
SEGS_PER_CORE = NUM_GRAPHS // N_CORES   # 2048
NGROUP = SEGS_PER_CORE // P             # 16 groups of 128 segs
NBLK = 4 * NGROUP                       # 64 blocks of 32 segs per core
NBIN = N_CORES * NBLK                   # 512 bins globally
LOOKAHEAD = 20                          # blocks of produce-ahead

TRACE = False
LAST_RESULT = None
_BUILD_CACHE = {}


def _build(T):
    from contextlib import ExitStack
    import concourse.bass as bass
    import concourse.bacc as bacc
    import concourse.tile as tile
    from concourse import mybir

    nc = bacc.Bacc("TRN2", target_bir_lowering=False, debug=False,
                   num_devices=N_CORES)
    dt = mybir.dt
    xq = nc.dram_tensor("xq", [P, NBLK * T * 512], dt.float8e3,
                        kind="ExternalInput").ap()
    srel = nc.dram_tensor("srel", [P, NBLK * T], dt.bfloat16,
                          kind="ExternalInput").ap()
    wb = nc.dram_tensor("wb", [IN_CH + 1, OUT_CH], dt.float16,
                        kind="ExternalInput").ap()
    scale = nc.dram_tensor("scale", [P, NGROUP], dt.float32,
                           kind="ExternalInput").ap()
    out = nc.dram_tensor("out", [SEGS_PER_CORE, OUT_CH], dt.float16,
                         kind="ExternalOutput").ap()

    def ap3(t_, off, d1, d2):
        return bass.AP(tensor=t_.tensor, offset=t_.offset + off,
                       ap=[t_.ap[0], d1, d2])

    def ap4(t_, off, d1, d2, d3):
        return bass.AP(tensor=t_.tensor, offset=t_.offset + off,
                       ap=[t_.ap[0], d1, d2, d3])

    with tile.TileContext(nc) as tc, ExitStack() as ctx:
        singles = ctx.enter_context(tc.tile_pool(name="singles", bufs=1))
        slabs = ctx.enter_context(
            tc.tile_pool(name="slabs", bufs=LOOKAHEAD // 2 + 2))
        ohpool = ctx.enter_context(
            tc.tile_pool(name="ohpool", bufs=LOOKAHEAD + 3))
        meanpool = ctx.enter_context(tc.tile_pool(name="meanpool", bufs=2))
        sbtpool = ctx.enter_context(tc.tile_pool(name="sbtpool", bufs=2))
        outpool = ctx.enter_context(tc.tile_pool(name="outpool", bufs=2))
        psum_acc = ctx.enter_context(
            tc.tile_pool(name="psum_acc", bufs=3, space="PSUM"))
        psum_t = ctx.enter_context(
            tc.tile_pool(name="psum_t", bufs=2, space="PSUM"))
        psum_o = ctx.enter_context(
            tc.tile_pool(name="psum_o", bufs=2, space="PSUM"))

        accs = {}

        def epilogue(g):
            acc = accs.pop(g)
            means = meanpool.tile([P, IN_CH + 1], dt.float16)
            nc.vector.tensor_scalar_mul(means[:, 0:IN_CH], acc,
                                        scale_sb[:, g:g + 1])
            nc.gpsimd.memset(means[:, IN_CH:IN_CH + 1], 1.0)
            pt = psum_t.tile([IN_CH + 1, P], dt.float16)
            nc.tensor.transpose(pt, means, ident_sb)
            sbt = sbtpool.tile([IN_CH + 1, P], dt.float16)
            nc.vector.tensor_copy(sbt, pt)
            po = psum_o.tile([P, OUT_CH], dt.float32)
            nc.tensor.matmul(po, lhsT=sbt, rhs=wb_sb, start=True, stop=True)
            osb = outpool.tile([P, OUT_CH], dt.float16)
            nc.vector.tensor_copy(osb, po)
            nc.gpsimd.dma_start(out[g * P:(g + 1) * P, :], osb)

        produced = {}
        oh_made = {}
        dma_i = 0

        def produce(sb):
            # one DMA per 2 blocks
            nonlocal dma_i
            if 2 * sb >= NBLK:
                return
            ring = (nc.sync, nc.gpsimd, nc.scalar)[dma_i % 3]
            dma_i += 1
            xs = slabs.tile([P, 2 * T * 512], dt.float8e3, name="xs")
            ring.dma_start(xs, xq[:, 2 * sb * T * 512:(2 * sb + 2) * T * 512])
            produced[sb] = xs

        def make_oh(b):
            if b >= NBLK:
                return
            oh = ohpool.tile([P, T * 32], dt.float8e3, name="oh")
            nc.vector.tensor_tensor(
                oh, iota_sb, ap3(srel_sb, b * T, [1, T], [0, 32]),
                mybir.AluOpType.is_equal)
            oh_made[b] = oh

        def consume(b):
            g_idx = b // 4
            strip = 32 * (b % 4)
            xs = produced[b // 2] if b % 2 == 0 else produced.pop(b // 2)
            off = (b % 2) * T * 512
            oh = oh_made.pop(b)
            acc = accs[g_idx]
            sl = acc[strip:strip + 32, :]
            dst = bass.AP(tensor=sl.tensor, offset=sl.offset,
                          ap=[sl.ap[0], [0, 8], [1, IN_CH]])
            for g in range(T):
                nc.tensor.matmul(
                    dst,
                    lhsT=oh[:, g * 32:(g + 1) * 32],
                    rhs=xs[:, off + g * 512:off + (g + 1) * 512],
                    start=(g == 0), stop=(g == T - 1),
                    tile_position=(0, strip))

        # PE warmup: dummy matmuls so HAM unthrottles before real work
        # arrives; deps only on a memset so they start immediately.
        wa = singles.tile([P, IN_CH], dt.float16, name="wa")
        nc.vector.memset(wa, 0.0)
        pw = psum_t.tile([32, IN_CH], dt.float32, name="pt")
        for r in range(56):
            nc.tensor.matmul(pw, lhsT=wa[:, 0:32], rhs=wa,
                             start=True, stop=True)

        # small DMAs are descriptor-bound (~45ns/partition-line): split the
        # load-bearing ones by partition across all three queues so they
        # land in ~2us instead of ~6us.
        srel_sb = singles.tile([P, NBLK * T], dt.bfloat16, name="srel")
        scale_sb = singles.tile([P, NGROUP], dt.float32, name="scale")
        for i, ring in enumerate((nc.sync, nc.scalar, nc.gpsimd)):
            p0, p1 = (P * i) // 3, (P * (i + 1)) // 3
            ring.dma_start(srel_sb[p0:p1, :], srel[p0:p1, :])
        for i, ring in enumerate((nc.sync, nc.scalar, nc.gpsimd)):
            p0, p1 = (P * i) // 3, (P * (i + 1)) // 3
            ring.dma_start(scale_sb[p0:p1, :], scale[p0:p1, :])
        wb_sb = singles.tile([IN_CH + 1, OUT_CH], dt.float16, name="wb")
        nc.scalar.dma_start(wb_sb, wb)
        # iota + identity built on device (gpsimd), no DMA at all
        iota_sb = singles.tile([P, T * 32], dt.bfloat16, name="iota")
        nc.gpsimd.iota(iota_sb, pattern=[[0, T], [1, 32]], base=0,
                       channel_multiplier=0,
                       allow_small_or_imprecise_dtypes=True)
        ident_sb = singles.tile([P, P], dt.float16, name="ident")
        nc.gpsimd.memset(ident_sb, 1.0)
        nc.gpsimd.affine_select(ident_sb, ident_sb, pattern=[[1, P]],
                                compare_op=mybir.AluOpType.is_equal,
                                fill=0.0, base=0, channel_multiplier=-1)
        for sb in range(LOOKAHEAD // 2):
            produce(sb)
        for b in range(min(LOOKAHEAD, NBLK)):
            make_oh(b)
        for g_idx in range(NGROUP):
            accs[g_idx] = psum_acc.tile([P, IN_CH], dt.float32, name="acc")
            for j in range(4):
                b = 4 * g_idx + j
                consume(b)
                if (b + LOOKAHEAD) % 2 == 0:
                    produce((b + LOOKAHEAD) // 2)
                make_oh(b + LOOKAHEAD)
            if g_idx >= 1:
                epilogue(g_idx - 1)
        epilogue(NGROUP - 1)
    nc.compile()
    return nc


def _sigma_delta_fp8(xpad, valid, qdtype):
    """Native-grid error-feedback quantization along axis 1."""
    S, L, F = xpad.shape
    q = np.zeros((S, L, F), qdtype)
    delta = np.zeros((S, F), np.float32)
    for j in range(L):
        m = valid[:, j][:, None]
        a = xpad[:, j, :] + delta
        qj = a.astype(qdtype)
        qf = qj.astype(np.float32)
        q[:, j, :] = np.where(m, qj, np.zeros((), qdtype))
        delta = np.where(m, a - qf, delta)
    return q


def _ensure_ntff_hook():
    import sys
    import types
    try:
        import antenv.axon_hooks  # noqa: F401
        return
    except ImportError:
        pass
    import antenv
    mod = types.ModuleType("antenv.axon_hooks")
    holder = {"h": None}
    mod.set_axon_ntff_profile_hook = lambda h: holder.__setitem__("h", h)
    mod.get_axon_ntff_profile_hook = lambda: holder["h"]
    sys.modules["antenv.axon_hooks"] = mod
    antenv.axon_hooks = mod
    try:
        from trn_agent_boot.trn_boot import _ntff_profile_via_ctypes
        mod.set_axon_ntff_profile_hook(
            _ntff_profile_via_ctypes("/opt/axon/libaxon_pjrt.so"))
    except Exception as e:
        print(f"ntff hook unavailable: {e}")


def kernel(x, segment_ids, weight, bias, num_graphs):
    global LAST_RESULT
    import ml_dtypes
    from concourse import bass_utils

    if TRACE:
        _ensure_ntff_hook()

    f8e4 = ml_dtypes.float8_e3m4
    bf16 = ml_dtypes.bfloat16
    x = np.asarray(x, dtype=np.float32)
    seg = np.asarray(segment_ids).astype(np.int64)
    weight = np.asarray(weight, dtype=np.float32)
    bias = np.asarray(bias, dtype=np.float32)
    G = int(num_graphs)
    assert G == NUM_GRAPHS and x.shape == (N_NODES, IN_CH)

    bounds = np.searchsorted(seg, np.arange(G + 1))
    cnts = np.diff(bounds).astype(np.int64)
    m = (cnts + 7) // 8                      # slot rows per seg

    # ---- snake-pack segments into 512 bins of exactly 32 segs ----
    order = np.argsort(-m, kind="stable")
    bin_of_seg = np.empty(G, np.int64)
    local_of_seg = np.empty(G, np.int64)
    fwd = np.arange(NBIN)
    rev = fwd[::-1]
    for r in range(G // NBIN):               # 32 rounds
        rowsegs = order[r * NBIN:(r + 1) * NBIN]
        bins = fwd if r % 2 == 0 else rev
        bin_of_seg[rowsegs] = bins
        local_of_seg[rowsegs] = r
    R = np.zeros(NBIN, np.int64)
    np.add.at(R, bin_of_seg, m)
    T = int(np.ceil(R.max() / P))
    assert T * P >= R.max()

    # per-seg starting slot row within its block (assignment order per bin)
    row_start = np.zeros(G, np.int64)
    base = np.zeros(NBIN, np.int64)
    for r in range(G // NBIN):
        rowsegs = order[r * NBIN:(r + 1) * NBIN]
        b = bin_of_seg[rowsegs]
        row_start[rowsegs] = base[b]
        base[b] += m[rowsegs]

    # ---- sigma-delta quantize on e4m3 grid ----
    L = int(m.max() * 8)
    idx_in_seg = np.arange(N_NODES) - bounds[seg]
    xpad = np.zeros((G, L, IN_CH), np.float32)
    vpad = np.zeros((G, L), bool)
    xpad[seg, idx_in_seg] = x
    vpad[seg, idx_in_seg] = True
    q = _sigma_delta_fp8(xpad, vpad, f8e4)   # [G, L, F]
    del xpad, vpad
    q = q.reshape(G, L // 8, 8, IN_CH)

    # ---- scatter into per-core slabs ----
    # per slot row: seg, row index within block
    seg_rep = np.repeat(np.arange(G), m)                    # [Rtot]
    csum = np.concatenate([[0], np.cumsum(m)])
    r_in_seg = np.arange(len(seg_rep)) - csum[seg_rep]      # [Rtot]
    row_blk = row_start[seg_rep] + r_in_seg                 # block row
    bin_r = bin_of_seg[seg_rep]
    core_r = bin_r // NBLK
    blk_r = bin_r % NBLK
    g_r = row_blk // P
    p_r = row_blk % P

    xq_all = np.zeros((N_CORES, P, NBLK * T * 512), f8e4)
    vals = q[seg_rep, r_in_seg]                             # [Rtot, 8, F]
    cols = (blk_r * (T * 512) + g_r * 512)[:, None, None] + \
        (np.arange(8) * IN_CH)[None, :, None] + \
        np.arange(IN_CH)[None, None, :]
    xq_all[core_r[:, None, None], p_r[:, None, None], cols] = vals
    del q, vals, cols

    srel_all = np.full((N_CORES, P, NBLK * T), -1.0, np.float32)
    srel_all[core_r, p_r, blk_r * T + g_r] = local_of_seg[seg_rep]

    # ---- epilogue scale + output permutation ----
    # device row (core, grp*128 + p) holds seg with bin=core*NBLK+grp*4+p//32,
    # local=p%32
    grp = np.arange(SEGS_PER_CORE) // P
    p_of = np.arange(SEGS_PER_CORE) % P
    seg_at = np.empty((N_CORES, SEGS_PER_CORE), np.int64)
    inv = np.empty(G, np.int64)
    inv[bin_of_seg * 32 + local_of_seg] = np.arange(G)
    for c in range(N_CORES):
        bins_ = c * NBLK + grp * 4 + p_of // 32
        seg_at[c] = inv[bins_ * 32 + p_of % 32]
    sc = 1.0 / np.maximum(cnts, 1).astype(np.float32)[seg_at]  # [C, 2048]
    scale_all = np.ascontiguousarray(
        sc.reshape(N_CORES, NGROUP, P).transpose(0, 2, 1)).astype(np.float32)

    wb = np.concatenate([weight.T, bias[None]], axis=0).astype(np.float16)

    if T not in _BUILD_CACHE:
        _BUILD_CACHE[T] = _build(T)
    nc = _BUILD_CACHE[T]

    in_maps = [
        dict(xq=xq_all[c], srel=srel_all[c].astype(bf16), wb=wb,
             scale=scale_all[c])
        for c in range(N_CORES)
    ]
    res = bass_utils.run_bass_kernel_spmd(
        nc, in_maps, core_ids=list(range(N_CORES)), trace=TRACE)
    LAST_RESULT = res
    dev = np.concatenate(
        [res.results[c]["out"] for c in range(N_CORES)], axis=0)
    out_full = np.empty((G, OUT_CH), np.float32)
    out_full[seg_at.reshape(-1)] = dev.astype(np.float32)
    return out_full
